# revision 1
# baseline (speedup 1.0000x reference)
"""Biaffine edge attention on 8 Trainium2 NeuronCores.

out[b,i,j] = head[b,i,:] @ edge_U @ dep[b,j,:] + head[b,i,:]@w1 + dep[b,j,:]@w2 + b0

Sharding: data-parallel over batch (B=8, one batch per core). Per core:
  HT = transpose(head[b])                (PE identity-transpose, fp32r)
  T1T[k,i] = sum_d U[d,k] * HT[d,i]      (fp32r matmul, lhsT=U natural layout)
  PT = transpose(dep[b])
  out[i,j] = sum_k T1T[k,i] * PT[k,j] + s_head[i] + s_dep[j] + b0

Matmuls/transposes run in float32r (full PE rate at free dim >= 512, ~fp32
precision). DMA loads go straight into fp32r tiles (verified numerically OK
on HW). Transposes of the second half of H / of P are interleaved into the
matmul instruction stream so they execute at the warm (2.4 GHz) PE clock --
isolated transpose-mode work does not trip the HAM un-throttle.
"""

import numpy as np

import concourse.bass as bass
import concourse.mybir as mybir
import concourse.tile as tile
from concourse import bacc
from concourse.bass_utils import run_bass_kernel_spmd
from concourse.masks import make_identity

B, S, D = 8, 1024, 1024
P = 128
SO = S // P  # 8
DO = D // P  # 8
NH = 512     # matmul free-dim tile (one fp32 PSUM bank)
F32 = mybir.dt.float32
F32R = mybir.dt.float32r
ADD = mybir.AluOpType.add
MULT = mybir.AluOpType.mult

_CACHE = {}


def build_nc(variant=4):
    nc = bacc.Bacc(None, target_bir_lowering=False)

    head = nc.dram_tensor("head", [S, D], F32R, kind="ExternalInput")
    dep = nc.dram_tensor("dep", [S, D], F32R, kind="ExternalInput")
    # host-relayouted U: u_prep[kt, dd, do, k] = U[do*P+dd, kt*P+k] so each
    # kt column-block is one contiguous 4KB chunk per partition
    edge_u = nc.dram_tensor("edge_u", [DO, P, DO, P], F32R, kind="ExternalInput")
    w_head_bc = nc.dram_tensor("w_head_bc", [P, D], F32, kind="ExternalInput")
    w_dep_col = nc.dram_tensor("w_dep_col", [P, DO], F32R, kind="ExternalInput")
    bias0 = nc.dram_tensor("bias0", [1, 1], F32, kind="ExternalInput")
    out = nc.dram_tensor("out", [S, S], F32, kind="ExternalOutput")

    with tile.TileContext(nc) as tc:
        with (
            tc.tile_pool(name="const", bufs=1) as const,
            tc.tile_pool(name="big", bufs=1) as big,
            tc.tile_pool(name="stage", bufs=8) as stage,
            tc.tile_pool(name="scratch", bufs=2) as scratch,
            tc.tile_pool(name="outp", bufs=4) as outp,
            tc.tile_pool(name="tp_ps", bufs=2, space="PSUM") as tp_ps,
            tc.tile_pool(name="mm_ps", bufs=5, space="PSUM") as mm_ps,
            tc.tile_pool(name="sm_ps", bufs=1, space="PSUM") as sm_ps,
        ):
            ident_raw = const.tile([P, P], F32)
            make_identity(nc, ident_raw)
            ident = const.tile([P, P], F32R)
            nc.vector.tensor_copy(ident[:], ident_raw[:])
            b_raw = const.tile([1, 1], F32)
            wd_sb = const.tile([P, DO], F32R)
            wh_sb = const.tile([P, D], F32)
            shead_col = const.tile([P, SO], F32)
            sdep_row = const.tile([1, S], F32)
            sdep_full = const.tile([P, S], F32)

            u_sb = big.tile([P, DO, D], F32R, tag="u")      # [dd, do, k]
            ht_sb = big.tile([P, DO, S], F32R, tag="ht")    # [dd, do, i]
            pt_sb = big.tile([P, DO, S], F32R, tag="pt")    # [kk, kt, j]
            t1t_sb = big.tile([P, DO, S], F32R, tag="t1t")  # [kk, kt, i]

            # ---------- DMA emission (sync ring is FIFO: order = priority) --
            h_stage = [None] * SO
            p_stage = [None] * SO

            def load_stage(src, arr, idx, split=1):
                t = stage.tile([P, D], F32R, tag="stage")
                w = D // split
                for s in range(split):
                    nc.sync.dma_start(
                        t[:, s * w:(s + 1) * w],
                        src[idx * P:(idx + 1) * P, s * w:(s + 1) * w],
                    )
                arr[idx] = t

            # All loads on the sync HWDGE ring (FIFO dispatch). U column-block
            # loads have expensive descriptor generation (~2-5 us dispatch), so
            # interleave them with the H stages to rate-match consumption:
            # phase A eats h0..h3, phase B eats one u column + one h stage per
            # kt group.
            def load_u_col(kt):
                nc.sync.dma_start(
                    u_sb[:, :, kt * P:(kt + 1) * P], edge_u[kt]
                )

            load_stage(head, h_stage, 0, split=2)
            for io in range(1, 4):
                load_stage(head, h_stage, io)
            load_u_col(0)
            load_u_col(1)
            load_u_col(2)
            for io in range(4, SO):
                load_stage(head, h_stage, io)
                load_u_col(io - 1)
            load_u_col(7)
            nc.sync.dma_start(wh_sb[:], w_head_bc[:])
            nc.sync.dma_start(wd_sb[:], w_dep_col[:])
            nc.sync.dma_start(b_raw[:], bias0[:])

            # ---------- helpers ----------
            copy_eng = [0]

            def copy(dst, src, eng=None):
                if eng is None:
                    eng = "act" if copy_eng[0] % 2 == 0 else "dve"
                    copy_eng[0] += 1
                if eng == "act":
                    nc.scalar.copy(dst, src)
                else:
                    nc.vector.tensor_copy(dst, src)

            def tpose_group(stages, idx, q4, dst_big, eng=None):
                """Transpose 4 [P,P] blocks (dims q4*4..q4*4+3) of stage idx."""
                ps = tp_ps.tile([P, NH], F32R, tag="tp")
                for q in range(4):
                    do = q4 * 4 + q
                    nc.tensor.transpose(
                        ps[:, q * P:(q + 1) * P],
                        stages[idx][:, do * P:(do + 1) * P],
                        ident[:],
                    )
                dst = dst_big[:, q4 * 4:q4 * 4 + 4, idx * P:(idx + 1) * P]
                copy(dst, ps[:].rearrange("p (q c) -> p q c", q=4), eng)

            def mm1_group(kt, ih, eng=None):
                ps = mm_ps.tile([P, NH], F32, tag="mm")
                for do in range(DO):
                    nc.tensor.matmul(
                        ps[:],
                        u_sb[:, do, kt * P:(kt + 1) * P],
                        ht_sb[:, do, ih * NH:(ih + 1) * NH],
                        start=(do == 0),
                        stop=(do == DO - 1),
                    )
                copy(t1t_sb[:, kt, ih * NH:(ih + 1) * NH], ps[:], eng)

            def shead_ops(io):
                sc = scratch.tile([P, D], F32, tag="scratch")
                nc.vector.tensor_mul(sc[:], h_stage[io][:].bitcast(F32), wh_sb[:])
                nc.vector.reduce_sum(
                    shead_col[:, io:io + 1], sc[:], axis=mybir.AxisListType.X
                )

            def sdep_ops(jh):
                ps = sm_ps.tile([P, NH], F32, tag="sm")
                for kt in range(DO):
                    nc.tensor.matmul(
                        ps[0:1, :],
                        wd_sb[:, kt:kt + 1],
                        pt_sb[:, kt, jh * NH:(jh + 1) * NH],
                        start=(kt == 0),
                        stop=(kt == DO - 1),
                    )
                nc.vector.tensor_scalar(
                    sdep_row[0:1, jh * NH:(jh + 1) * NH],
                    ps[0:1, :], b_raw[0:1, 0:1], None, ADD,
                )
                nc.gpsimd.partition_broadcast(
                    sdep_full[:, jh * NH:(jh + 1) * NH],
                    sdep_row[0:1, jh * NH:(jh + 1) * NH],
                )

            def mm2_group(it, jh, split=1):
                ps = mm_ps.tile([P, NH], F32, tag="mm")
                for kt in range(DO):
                    nc.tensor.matmul(
                        ps[:],
                        t1t_sb[:, kt, it * P:(it + 1) * P],
                        pt_sb[:, kt, jh * NH:(jh + 1) * NH],
                        start=(kt == 0),
                        stop=(kt == DO - 1),
                    )
                ot = outp.tile([P, NH], F32, tag="out")
                w = NH // split
                for s in range(split):
                    sl = slice(s * w, (s + 1) * w)
                    nc.vector.scalar_tensor_tensor(
                        out=ot[:, sl], in0=ps[:, sl],
                        scalar=shead_col[:, it:it + 1],
                        in1=sdep_full[:, jh * NH + s * w:jh * NH + (s + 1) * w],
                        op0=ADD, op1=ADD,
                    )
                    nc.sync.dma_start(
                        out[it * P:(it + 1) * P,
                            jh * NH + s * w:jh * NH + (s + 1) * w],
                        ot[:, sl],
                    )

            # ---------- phase A: transpose H rows io 0..3 ----------
            for io in range(4):
                for q4 in range(2):
                    tpose_group(h_stage, io, q4, ht_sb)

            # ---------- phase B: mm1 ih=0, interleave H transposes io 4..7 --
            pend = [(io, q4) for io in range(4, SO) for q4 in range(2)]
            for kt in range(DO):
                if kt >= DO - 2:
                    io, q4 = pend.pop(0)
                    tpose_group(h_stage, io, q4, ht_sb)
                mm1_group(kt, 0)
                if kt < DO - 2:
                    io, q4 = pend.pop(0)
                    tpose_group(h_stage, io, q4, ht_sb)

            # s_head on DVE (after phase-B copies in DVE program order, so the
            # early transpose-copy drain is not blocked behind the wh_sb DMA)
            for io in range(SO):
                shead_ops(io)

            # ---------- P loads (reuse stage slots as they free up) ----------
            for jo in range(SO):
                load_stage(dep, p_stage, jo)

            # ---------- phase C: mm1 ih=1, interleave P transposes jo 0..3 --
            # all copies on ACT: DVE is busy with the s_head mult/reduce block
            pend = [(jo, q4) for jo in range(4) for q4 in range(2)]
            for kt in range(DO):
                mm1_group(kt, 1, eng="act")
                jo, q4 = pend.pop(0)
                tpose_group(p_stage, jo, q4, pt_sb, eng="act")

            # ---------- phase D/E: sdep half 0, mm2 jh=0 + P transposes 4..7
            sdep_ops(0)
            pend = [(jo, q4) for jo in range(4, SO) for q4 in range(2)]
            for it in range(SO):
                mm2_group(it, 0)
                jo, q4 = pend.pop(0)
                tpose_group(p_stage, jo, q4, pt_sb, eng="act")

            # ---------- phase F/G: sdep half 1, mm2 jh=1 ----------
            sdep_ops(1)
            for it in range(SO):
                # split the last group's epilogue so the tail latency chain
                # (STT -> out DMA) is half as long
                mm2_group(it, 1, split=(4 if it == SO - 1 else 1))

    nc.compile()
    return nc


def _get_nc(variant=4):
    key = ("nc", variant)
    if key not in _CACHE:
        _CACHE[key] = build_nc(variant)
    return _CACHE[key]


def _in_maps(head, dep, edge_U, edge_W, edge_b):
    # pull everything to host numpy first (inputs may be jax device arrays)
    head = np.asarray(head, dtype=np.float32)
    dep = np.asarray(dep, dtype=np.float32)
    edge_U = np.asarray(edge_U, dtype=np.float32)
    w = np.asarray(edge_W, dtype=np.float32).reshape(-1)
    w1, w2 = w[:D], w[D:]
    w_head_bc = np.ascontiguousarray(np.broadcast_to(w1[None, :], (P, D)))
    w_dep_col = np.ascontiguousarray(w2.reshape(DO, P).T)  # [kk, kt]
    b0 = np.asarray(edge_b, dtype=np.float32).reshape(1, 1)
    u_prep = np.ascontiguousarray(
        np.asarray(edge_U, dtype=np.float32)
        .reshape(DO, P, DO, P).transpose(2, 1, 0, 3)
    )
    maps = []
    for b in range(B):
        maps.append({
            "head": np.ascontiguousarray(head[b], dtype=np.float32),
            "dep": np.ascontiguousarray(dep[b], dtype=np.float32),
            "edge_u": u_prep,
            "w_head_bc": w_head_bc,
            "w_dep_col": w_dep_col,
            "bias0": b0,
        })
    return maps


def kernel(head, dep, edge_U, edge_W, edge_b, **run_kwargs):
    nc = _get_nc()
    maps = _in_maps(head, dep, edge_U, edge_W, edge_b)
    res = run_bass_kernel_spmd(nc, maps, core_ids=list(range(B)), **run_kwargs)
    out = np.stack([res.results[c]["out"] for c in range(B)], axis=0)
    if run_kwargs:
        _CACHE["last_result"] = res
    return out



# revision 4
# speedup vs baseline: 1.2095x; 1.2095x over previous
"""Biaffine edge attention on 8 Trainium2 NeuronCores.

out[b,i,j] = head[b,i,:] @ edge_U @ dep[b,j,:] + head[b,i,:]@w1 + dep[b,j,:]@w2 + b0

Sharding: data-parallel over batch (B=8, one batch per core).

Layout strategy: head/dep are transposed on the host (pure relayout, like the
baseline's U relayout) so the device does ZERO PE transposes:
  HT[d,i] = head[b,i,d],  PT[k,j] = dep[b,j,k]   (bf16, [dt, 128, S] blocks)
  mm1: T1T[k,i] = sum_d U[d,k] HT[d,i]   lhsT = U row-block (natural layout)
  mm2: out[i,j] = sum_k T1T[k,i] PT[k,j] lhsT = T1T (mm1's natural output)

All matmul operands are bf16: same PE rate as fp32r (1 cycle/row) but half
the DMA traffic and SBUF footprint. PSUM accumulates fp32; rel err ~5e-3.

mm1 runs dt-outer across all 8 PSUM banks so compute starts after only
U row-block 0 + HT block 0 (~0.5 MB) instead of the full 2.25 MB. A short
burst of identity matmuls warms the PE clock (HAM ramps 1.2->2.4 GHz after
~3 us of continuous work) inside the initial DMA shadow.

s_head/s_dep are computed on-device as [1,S] row matmuls; s_head is flipped
to a per-partition column with 8 tiny transposes; bias folds into s_head.
Epilogue: DVE scalar_tensor_tensor -> bf16 out tile -> DMA (host casts back
to fp32).
"""

import numpy as np
import ml_dtypes

import concourse.bass as bass
import concourse.mybir as mybir
import concourse.tile as tile
from concourse import bacc
from concourse.bass_utils import run_bass_kernel_spmd
from concourse.masks import make_identity

B, S, D = 8, 1024, 1024
P = 128
DO = D // P  # 8
NH = 512     # fp32 PSUM bank free size
NWARM = 16
F32 = mybir.dt.float32
BF16 = mybir.dt.bfloat16
ADD = mybir.AluOpType.add

_CACHE = {}


def build_nc(nwarm=NWARM):
    nc = bacc.Bacc(None, target_bir_lowering=False)

    # host-pretransposed inputs, all bf16
    ht = nc.dram_tensor("ht", [DO, P, S], BF16, kind="ExternalInput")   # [dt, dd, i]
    pt = nc.dram_tensor("pt", [DO, P, S], BF16, kind="ExternalInput")   # [kt, kk, j]
    u = nc.dram_tensor("u", [DO, P, D], BF16, kind="ExternalInput")     # [dt, dd, k]
    wc = nc.dram_tensor("wc", [P, 2 * DO], BF16, kind="ExternalInput")  # w1|w2 cols
    bias0 = nc.dram_tensor("bias0", [1, 1], F32, kind="ExternalInput")
    out = nc.dram_tensor("out", [S, S], BF16, kind="ExternalOutput")

    with tile.TileContext(nc) as tc:
        with (
            tc.tile_pool(name="const", bufs=1) as const,
            tc.tile_pool(name="big", bufs=1) as big,
            tc.tile_pool(name="outp", bufs=4) as outp,
            tc.tile_pool(name="ps", bufs=8, space="PSUM") as psp,
        ):
            ident = const.tile([P, P], F32)
            make_identity(nc, ident)
            ident_b = const.tile([P, P], BF16)
            nc.vector.tensor_copy(ident_b[:], ident[:])

            wc_sb = const.tile([P, 2 * DO], BF16)
            b_sb = const.tile([1, 1], F32)
            shead_col = const.tile([P, DO], F32)
            row_sb = const.tile([1, S], F32)     # s_head + bias
            drow_sb = const.tile([1, S], F32)    # s_dep
            sdep_full = const.tile([P, S], F32)

            u_sb = big.tile([P, DO, D], BF16, tag="u")      # [dd, dt, k]
            ht_sb = big.tile([P, DO, S], BF16, tag="ht")    # [dd, dt, i]
            pt_sb = big.tile([P, DO, S], BF16, tag="pt")    # [kk, kt, j]
            t1t_sb = big.tile([P, DO, S], BF16, tag="t1t")  # [kk, kt, i]

            # ---------- DMA emission (sync ring is FIFO: order = priority) --
            nc.sync.dma_start(wc_sb[:], wc[:])
            nc.sync.dma_start(b_sb[:], bias0[:])
            for dt in range(DO):
                nc.sync.dma_start(u_sb[:, dt, :], u[dt])
                nc.sync.dma_start(ht_sb[:, dt, :], ht[dt])
            for kt in range(DO):
                nc.sync.dma_start(pt_sb[:, kt, :], pt[kt])

            # ---------- PE warmup: real matmuls inside the DMA shadow -------
            warm_ps = psp.tile([P, NH], F32, tag="ps")
            for _ in range(nwarm):
                nc.tensor.matmul(
                    warm_ps[:, 0:P], ident_b[:], ident_b[:], start=True, stop=True
                )

            copy_i = [0]

            def copy(dst, src):
                if copy_i[0] % 2 == 0:
                    nc.scalar.copy(dst, src)
                else:
                    nc.vector.tensor_copy(dst, src)
                copy_i[0] += 1

            # ---------- mm1 (dt-outer over all 8 PSUM banks) ----------------
            for ih in range(2):
                ps1 = [
                    psp.tile([P, NH], F32, tag="ps", name=f"ps1_{ih}_{k}")
                    for k in range(DO)
                ]
                for dt in range(DO):
                    for kt in range(DO):
                        nc.tensor.matmul(
                            ps1[kt][:],
                            u_sb[:, dt, kt * P:(kt + 1) * P],
                            ht_sb[:, dt, ih * NH:(ih + 1) * NH],
                            start=(dt == 0),
                            stop=(dt == DO - 1),
                        )
                for kt in range(DO):
                    copy(t1t_sb[:, kt, ih * NH:(ih + 1) * NH], ps1[kt][:])

            # ---------- s_head row (+bias) -> columns -----------------------
            for ih in range(2):
                ps_r = psp.tile([P, NH], F32, tag="ps")
                for dt in range(DO):
                    nc.tensor.matmul(
                        ps_r[0:1, :],
                        wc_sb[:, dt:dt + 1],
                        ht_sb[:, dt, ih * NH:(ih + 1) * NH],
                        start=(dt == 0),
                        stop=(dt == DO - 1),
                    )
                nc.vector.tensor_scalar(
                    row_sb[0:1, ih * NH:(ih + 1) * NH],
                    ps_r[0:1, :], b_sb[0:1, 0:1], None, ADD,
                )
            ps_c = psp.tile([P, NH], F32, tag="ps")
            for it in range(DO):
                nc.tensor.transpose(
                    ps_c[:, it:it + 1],
                    row_sb[0:1, it * P:(it + 1) * P],
                    ident[0:1, 0:1],
                )
            nc.scalar.copy(shead_col[:], ps_c[:, 0:DO])

            # ---------- s_dep row -> broadcast to all partitions ------------
            for jh in range(2):
                ps_d = psp.tile([P, NH], F32, tag="ps")
                for kt in range(DO):
                    nc.tensor.matmul(
                        ps_d[0:1, :],
                        wc_sb[:, DO + kt:DO + kt + 1],
                        pt_sb[:, kt, jh * NH:(jh + 1) * NH],
                        start=(kt == 0),
                        stop=(kt == DO - 1),
                    )
                nc.vector.tensor_copy(
                    drow_sb[0:1, jh * NH:(jh + 1) * NH], ps_d[0:1, :]
                )
                nc.gpsimd.partition_broadcast(
                    sdep_full[:, jh * NH:(jh + 1) * NH],
                    drow_sb[0:1, jh * NH:(jh + 1) * NH],
                )

            # ---------- mm2 + epilogue --------------------------------------
            for jh in range(2):
                for it in range(DO):
                    ps = psp.tile([P, NH], F32, tag="ps")
                    for kt in range(DO):
                        nc.tensor.matmul(
                            ps[:],
                            t1t_sb[:, kt, it * P:(it + 1) * P],
                            pt_sb[:, kt, jh * NH:(jh + 1) * NH],
                            start=(kt == 0),
                            stop=(kt == DO - 1),
                        )
                    # split the very last epilogue so the tail chain is short
                    split = 2 if (jh == 1 and it == DO - 1) else 1
                    ot = outp.tile([P, NH], BF16, tag="out")
                    w = NH // split
                    for s_ in range(split):
                        sl = slice(s_ * w, (s_ + 1) * w)
                        nc.vector.scalar_tensor_tensor(
                            out=ot[:, sl], in0=ps[:, sl],
                            scalar=shead_col[:, it:it + 1],
                            in1=sdep_full[:, jh * NH + s_ * w:jh * NH + (s_ + 1) * w],
                            op0=ADD, op1=ADD,
                        )
                        nc.sync.dma_start(
                            out[it * P:(it + 1) * P,
                                jh * NH + s_ * w:jh * NH + (s_ + 1) * w],
                            ot[:, sl],
                        )

    nc.compile()
    return nc


def _get_nc(nwarm=NWARM):
    key = ("nc", nwarm)
    if key not in _CACHE:
        _CACHE[key] = build_nc(nwarm)
    return _CACHE[key]


def _in_maps(head, dep, edge_U, edge_W, edge_b):
    bf16 = ml_dtypes.bfloat16
    head = np.asarray(head, dtype=np.float32)
    dep = np.asarray(dep, dtype=np.float32)
    u_prep = np.ascontiguousarray(
        np.asarray(edge_U, dtype=np.float32)
    ).astype(bf16).reshape(DO, P, D)
    w = np.asarray(edge_W, dtype=np.float32).reshape(-1)
    w1c = w[:D].reshape(DO, P).T
    w2c = w[D:].reshape(DO, P).T
    wc = np.ascontiguousarray(np.concatenate([w1c, w2c], axis=1)).astype(bf16)
    b0 = np.asarray(edge_b, dtype=np.float32).reshape(1, 1)
    head_b = head.astype(bf16)
    dep_b = dep.astype(bf16)
    maps = []
    for b in range(B):
        maps.append({
            "ht": np.ascontiguousarray(head_b[b].T).reshape(DO, P, S),
            "pt": np.ascontiguousarray(dep_b[b].T).reshape(DO, P, S),
            "u": u_prep,
            "wc": wc,
            "bias0": b0,
        })
    return maps


def kernel(head, dep, edge_U, edge_W, edge_b, **run_kwargs):
    nc = _get_nc()
    maps = _in_maps(head, dep, edge_U, edge_W, edge_b)
    res = run_bass_kernel_spmd(nc, maps, core_ids=list(range(B)), **run_kwargs)
    out = np.stack(
        [np.asarray(res.results[c]["out"]).astype(np.float32) for c in range(B)],
        axis=0,
    )
    if run_kwargs:
        _CACHE["last_result"] = res
    return out


# revision 5
# speedup vs baseline: 1.2119x; 1.0020x over previous
"""Biaffine edge attention on 8 Trainium2 NeuronCores.

out[b,i,j] = head[b,i,:] @ edge_U @ dep[b,j,:] + head[b,i,:]@w1 + dep[b,j,:]@w2 + b0

Sharding: data-parallel over batch (B=8, one batch per core).

Layout strategy: head/dep are transposed on the host (pure relayout, like the
baseline's U relayout) so the device does ZERO PE transposes:
  HT[d,i] = head[b,i,d],  PT[k,j] = dep[b,j,k]   (bf16, per-128-row blocks)
  mm1: T1T[k,i] = sum_d U[d,k] HT[d,i]   lhsT = U row-block (natural layout)
  mm2: out[i,j] = sum_k T1T[k,i] PT[k,j] lhsT = T1T (mm1's natural output)

All matmul operands are bf16: same PE rate as fp32r (1 cycle/row) but half
the DMA traffic and SBUF footprint. PSUM accumulates fp32; rel err ~4e-3.

Every 128-row block lives in its OWN SBUF tile so DMA->matmul dependencies
are exact (slices of one big tile made mm1 wait on unrelated later DMAs).
mm1 runs dt-outer across all 8 PSUM banks so round dt needs only u[dt] +
ht[dt]. A DMA'd bf16 identity feeds warmup matmuls that keep the PE busy
from the end of the ~7us framework preamble until the first data lands --
any PE idle gap resets the HAM clock ramp (1.2 GHz for ~6us after a gap).

s_head/s_dep are [1,S] row matmuls (all four back-to-back on PE), s_head is
flipped to a per-partition column with 8 tiny transposes, bias folds into
s_head. Epilogue: DVE scalar_tensor_tensor -> bf16 out tile -> DMA (host
casts back to fp32). The last mm2 group is column-split so the tail chain
(matmul -> STT -> out DMA) is half length.
"""

import numpy as np
import ml_dtypes

import concourse.bass as bass
import concourse.mybir as mybir
import concourse.tile as tile
from concourse import bacc
from concourse.bass_utils import run_bass_kernel_spmd

B, S, D = 8, 1024, 1024
P = 128
DO = D // P  # 8
NH = 512     # fp32 PSUM bank free size
NWARM = 12
F32 = mybir.dt.float32
BF16 = mybir.dt.bfloat16
ADD = mybir.AluOpType.add

_CACHE = {}


def build_nc(nwarm=NWARM):
    nc = bacc.Bacc(None, target_bir_lowering=False)

    # host-pretransposed inputs, all bf16
    identb = nc.dram_tensor("identb", [P, P], BF16, kind="ExternalInput")
    ht = nc.dram_tensor("ht", [DO, P, S], BF16, kind="ExternalInput")   # [dt, dd, i]
    pt = nc.dram_tensor("pt", [DO, P, S], BF16, kind="ExternalInput")   # [kt, kk, j]
    u = nc.dram_tensor("u", [DO, P, D], BF16, kind="ExternalInput")     # [dt, dd, k]
    wc = nc.dram_tensor("wc", [P, 2 * DO], BF16, kind="ExternalInput")  # w1|w2 cols
    bias0 = nc.dram_tensor("bias0", [1, 1], F32, kind="ExternalInput")
    out = nc.dram_tensor("out", [S, S], BF16, kind="ExternalOutput")

    with tile.TileContext(nc) as tc:
        with (
            tc.tile_pool(name="const", bufs=1) as const,
            tc.tile_pool(name="big", bufs=1) as big,
            tc.tile_pool(name="outp", bufs=4) as outp,
            tc.tile_pool(name="ps", bufs=8, space="PSUM") as psp,
        ):
            idb = const.tile([P, P], BF16)
            one_sb = const.tile([1, 1], F32)
            nc.gpsimd.memset(one_sb[:], 1.0)

            wc_sb = const.tile([P, 2 * DO], BF16)
            b_sb = const.tile([1, 1], F32)
            shead_col = const.tile([P, DO], F32)
            row_sb = const.tile([1, S], F32)     # s_head + bias
            drow_sb = const.tile([1, S], F32)    # s_dep
            sdep_full = const.tile([P, S], F32)

            u_t = [big.tile([P, D], BF16, tag=f"u{i}", name=f"u{i}")
                   for i in range(DO)]
            ht_t = [big.tile([P, S], BF16, tag=f"ht{i}", name=f"ht{i}")
                    for i in range(DO)]
            pt_t = [big.tile([P, S], BF16, tag=f"pt{i}", name=f"pt{i}")
                    for i in range(DO)]
            t1t_t = [big.tile([P, S], BF16, tag=f"t1t{i}", name=f"t1t{i}")
                     for i in range(DO)]

            # ---------- DMA emission (sync ring is FIFO: order = priority) --
            nc.sync.dma_start(idb[:], identb[:])
            for dt in range(DO):
                nc.sync.dma_start(u_t[dt][:], u[dt])
                nc.sync.dma_start(ht_t[dt][:], ht[dt])
            for kt in range(DO):
                nc.sync.dma_start(pt_t[kt][:], pt[kt])
            nc.sync.dma_start(wc_sb[:], wc[:])
            nc.sync.dma_start(b_sb[:], bias0[:])

            # ---------- PE warmup: real matmuls inside the DMA shadow -------
            warm_ps = psp.tile([P, NH], F32, tag="ps")
            for _ in range(nwarm):
                nc.tensor.matmul(
                    warm_ps[:, 0:P], idb[:], idb[:], start=True, stop=True
                )

            copy_i = [0]

            def copy(dst, src):
                if copy_i[0] % 2 == 0:
                    nc.scalar.copy(dst, src)
                else:
                    nc.vector.tensor_copy(dst, src)
                copy_i[0] += 1

            # ---------- mm1 (dt-outer over all 8 PSUM banks) ----------------
            for ih in range(2):
                ps1 = [
                    psp.tile([P, NH], F32, tag="ps", name=f"ps1_{ih}_{k}")
                    for k in range(DO)
                ]
                for dt in range(DO):
                    for kt in range(DO):
                        nc.tensor.matmul(
                            ps1[kt][:],
                            u_t[dt][:, kt * P:(kt + 1) * P],
                            ht_t[dt][:, ih * NH:(ih + 1) * NH],
                            start=(dt == 0),
                            stop=(dt == DO - 1),
                        )
                for kt in range(DO):
                    copy(t1t_t[kt][:, ih * NH:(ih + 1) * NH], ps1[kt][:])

            # ---------- s_head / s_dep rows: all PE matmuls back-to-back ----
            ps_r = []
            for ih in range(2):
                ps_ri = psp.tile([P, NH], F32, tag="ps", name=f"ps_r{ih}")
                for dt in range(DO):
                    nc.tensor.matmul(
                        ps_ri[0:1, :],
                        wc_sb[:, dt:dt + 1],
                        ht_t[dt][:, ih * NH:(ih + 1) * NH],
                        start=(dt == 0),
                        stop=(dt == DO - 1),
                    )
                nc.vector.tensor_scalar(
                    row_sb[0:1, ih * NH:(ih + 1) * NH],
                    ps_ri[0:1, :], b_sb[0:1, 0:1], None, ADD,
                )
                ps_r.append(ps_ri)
            for jh in range(2):
                ps_d = psp.tile([P, NH], F32, tag="ps", name=f"ps_d{jh}")
                for kt in range(DO):
                    nc.tensor.matmul(
                        ps_d[0:1, :],
                        wc_sb[:, DO + kt:DO + kt + 1],
                        pt_t[kt][:, jh * NH:(jh + 1) * NH],
                        start=(kt == 0),
                        stop=(kt == DO - 1),
                    )
                nc.vector.tensor_copy(
                    drow_sb[0:1, jh * NH:(jh + 1) * NH], ps_d[0:1, :]
                )
                nc.gpsimd.partition_broadcast(
                    sdep_full[:, jh * NH:(jh + 1) * NH],
                    drow_sb[0:1, jh * NH:(jh + 1) * NH],
                )
            # s_head row -> per-partition column (8 tiny PE transposes)
            ps_c = psp.tile([P, NH], F32, tag="ps")
            for it in range(DO):
                nc.tensor.transpose(
                    ps_c[:, it:it + 1],
                    row_sb[0:1, it * P:(it + 1) * P],
                    one_sb[0:1, 0:1],
                )
            nc.scalar.copy(shead_col[:], ps_c[:, 0:DO])

            # ---------- mm2 + epilogue --------------------------------------
            def mm2_group(it, jh, c0, c1):
                ps = psp.tile([P, c1 - c0], F32, tag="ps", name=f"mm2_{it}_{jh}")
                for kt in range(DO):
                    nc.tensor.matmul(
                        ps[:],
                        t1t_t[kt][:, it * P:(it + 1) * P],
                        pt_t[kt][:, jh * NH + c0:jh * NH + c1],
                        start=(kt == 0),
                        stop=(kt == DO - 1),
                    )
                ot = outp.tile([P, c1 - c0], BF16, tag="out", name=f"ot_{it}_{jh}_{c0}")
                nc.vector.scalar_tensor_tensor(
                    out=ot[:], in0=ps[:],
                    scalar=shead_col[:, it:it + 1],
                    in1=sdep_full[:, jh * NH + c0:jh * NH + c1],
                    op0=ADD, op1=ADD,
                )
                nc.sync.dma_start(
                    out[it * P:(it + 1) * P, jh * NH + c0:jh * NH + c1], ot[:]
                )

            for jh in range(2):
                for it in range(DO):
                    if jh == 1 and it == DO - 1:
                        # split the final group so the tail chain is short
                        mm2_group(it, jh, 0, NH // 2)
                        mm2_group(it, jh, NH // 2, NH)
                    else:
                        mm2_group(it, jh, 0, NH)

    nc.compile()
    return nc


def _get_nc(nwarm=NWARM):
    key = ("nc", nwarm)
    if key not in _CACHE:
        _CACHE[key] = build_nc(nwarm)
    return _CACHE[key]


def _in_maps(head, dep, edge_U, edge_W, edge_b):
    bf16 = ml_dtypes.bfloat16
    head = np.asarray(head, dtype=np.float32)
    dep = np.asarray(dep, dtype=np.float32)
    identb = np.eye(P, dtype=bf16)
    u_prep = np.ascontiguousarray(
        np.asarray(edge_U, dtype=np.float32)
    ).astype(bf16).reshape(DO, P, D)
    w = np.asarray(edge_W, dtype=np.float32).reshape(-1)
    w1c = w[:D].reshape(DO, P).T
    w2c = w[D:].reshape(DO, P).T
    wc = np.ascontiguousarray(np.concatenate([w1c, w2c], axis=1)).astype(bf16)
    b0 = np.asarray(edge_b, dtype=np.float32).reshape(1, 1)
    head_b = head.astype(bf16)
    dep_b = dep.astype(bf16)
    maps = []
    for b in range(B):
        maps.append({
            "identb": identb,
            "ht": np.ascontiguousarray(head_b[b].T).reshape(DO, P, S),
            "pt": np.ascontiguousarray(dep_b[b].T).reshape(DO, P, S),
            "u": u_prep,
            "wc": wc,
            "bias0": b0,
        })
    return maps


def kernel(head, dep, edge_U, edge_W, edge_b, **run_kwargs):
    nc = _get_nc()
    maps = _in_maps(head, dep, edge_U, edge_W, edge_b)
    res = run_bass_kernel_spmd(nc, maps, core_ids=list(range(B)), **run_kwargs)
    out = np.stack(
        [np.asarray(res.results[c]["out"]).astype(np.float32) for c in range(B)],
        axis=0,
    )
    if run_kwargs:
        _CACHE["last_result"] = res
    return out


# revision 7
# speedup vs baseline: 1.2512x; 1.0324x over previous
"""Biaffine edge attention on 8 Trainium2 NeuronCores.

out[b,i,j] = head[b,i,:] @ edge_U @ dep[b,j,:] + head[b,i,:]@w1 + dep[b,j,:]@w2 + b0

Sharding: data-parallel over batch (B=8, one batch per core).

Layout strategy: head/dep are transposed on the host (pure relayout, like the
baseline's U relayout) so the device does ZERO PE transposes:
  HT[d,i] = head[b,i,d],  PT[k,j] = dep[b,j,k]   (bf16, per-128-row blocks)
  mm1: T1T[k,i] = sum_d U[d,k] HT[d,i]   lhsT = U row-block (natural layout)
  mm2: out[i,j] = sum_k T1T[k,i] PT[k,j] lhsT = T1T (mm1's natural output)

All matmul operands are bf16: same PE rate as fp32r (1 cycle/row) but half
the DMA traffic and SBUF footprint. PSUM accumulates fp32; rel err ~4e-3.

Every 128-row block lives in its OWN SBUF tile so DMA->matmul dependencies
are exact (slices of one big tile made mm1 wait on unrelated later DMAs).
mm1 runs dt-outer across all 8 PSUM banks so round dt needs only u[dt] +
ht[dt]. A DMA'd bf16 identity feeds warmup matmuls that keep the PE busy
from the end of the ~7us framework preamble until the first data lands --
any PE idle gap resets the HAM clock ramp (1.2 GHz for ~6us after a gap).

s_head/s_dep are [1,S] row matmuls (all four back-to-back on PE), s_head is
flipped to a per-partition column with 8 tiny transposes, bias folds into
s_head. Epilogue: DVE scalar_tensor_tensor -> bf16 out tile -> DMA (host
casts back to fp32). The last mm2 group is column-split so the tail chain
(matmul -> STT -> out DMA) is half length.
"""

import numpy as np
import ml_dtypes

import concourse.bass as bass
import concourse.mybir as mybir
import concourse.tile as tile
from concourse import bacc
from concourse.bass_utils import run_bass_kernel_spmd

B, S, D = 8, 1024, 1024
P = 128
DO = D // P  # 8
NH = 512     # fp32 PSUM bank free size
NWARM = 28
F32 = mybir.dt.float32
BF16 = mybir.dt.bfloat16
ADD = mybir.AluOpType.add

_CACHE = {}


def build_nc(nwarm=NWARM):
    nc = bacc.Bacc(None, target_bir_lowering=False)

    # host-pretransposed inputs, all bf16
    identb = nc.dram_tensor("identb", [P, P], BF16, kind="ExternalInput")
    ht = nc.dram_tensor("ht", [DO, P, S], BF16, kind="ExternalInput")   # [dt, dd, i]
    pt = nc.dram_tensor("pt", [DO, P, S], BF16, kind="ExternalInput")   # [kt, kk, j]
    u = nc.dram_tensor("u", [DO, P, D], BF16, kind="ExternalInput")     # [dt, dd, k]
    wc = nc.dram_tensor("wc", [P, 2 * DO], BF16, kind="ExternalInput")  # w1|w2 cols
    bias0 = nc.dram_tensor("bias0", [1, 1], F32, kind="ExternalInput")
    out = nc.dram_tensor("out", [S, S], BF16, kind="ExternalOutput")

    with tile.TileContext(nc) as tc:
        with (
            tc.tile_pool(name="const", bufs=1) as const,
            tc.tile_pool(name="big", bufs=1) as big,
            tc.tile_pool(name="outp", bufs=4) as outp,
            tc.tile_pool(name="ps", bufs=8, space="PSUM") as psp,
        ):
            idb = const.tile([P, P], BF16)
            one_sb = const.tile([1, 1], F32)
            nc.gpsimd.memset(one_sb[:], 1.0)

            wc_sb = const.tile([P, 2 * DO], BF16)
            b_sb = const.tile([1, 1], F32)
            shead_col = const.tile([P, DO], F32)
            row_sb = const.tile([1, S], F32)     # s_head + bias
            drow_sb = const.tile([1, S], F32)    # s_dep
            sdep_full = const.tile([P, S], F32)

            u_t = [big.tile([P, D], BF16, tag=f"u{i}", name=f"u{i}")
                   for i in range(DO)]
            ht_t = [big.tile([P, S], BF16, tag=f"ht{i}", name=f"ht{i}")
                    for i in range(DO)]
            pt_t = [big.tile([P, S], BF16, tag=f"pt{i}", name=f"pt{i}")
                    for i in range(DO)]
            t1t_t = [big.tile([P, S], BF16, tag=f"t1t{i}", name=f"t1t{i}")
                     for i in range(DO)]

            # ---------- DMA emission (sync ring is FIFO: order = priority) --
            # mm1-ih0 needs u[dt] + left half of ht[dt]: stream those first,
            # then the right ht halves (for ih1), then pt / w / bias.
            nc.sync.dma_start(idb[:], identb[:])
            for dt in range(DO):
                nc.sync.dma_start(u_t[dt][:], u[dt])
                nc.sync.dma_start(ht_t[dt][:, 0:NH], ht[dt][:, 0:NH])
            for dt in range(DO):
                nc.sync.dma_start(ht_t[dt][:, NH:S], ht[dt][:, NH:S])
            for kt in range(DO):
                nc.sync.dma_start(pt_t[kt][:], pt[kt])
            nc.sync.dma_start(wc_sb[:], wc[:])
            nc.sync.dma_start(b_sb[:], bias0[:])

            # ---------- PE warmup: real matmuls inside the DMA shadow -------
            warm_ps = psp.tile([P, NH], F32, tag="ps")
            for _ in range(nwarm):
                nc.tensor.matmul(
                    warm_ps[:, 0:P], idb[:], idb[:], start=True, stop=True
                )

            copy_i = [0]

            def copy(dst, src):
                if copy_i[0] % 2 == 0:
                    nc.scalar.copy(dst, src)
                else:
                    nc.vector.tensor_copy(dst, src)
                copy_i[0] += 1

            # ---------- mm1 (dt-outer over all 8 PSUM banks) ----------------
            for ih in range(2):
                ps1 = [
                    psp.tile([P, NH], F32, tag="ps", name=f"ps1_{ih}_{k}")
                    for k in range(DO)
                ]
                for dt in range(DO):
                    for kt in range(DO):
                        nc.tensor.matmul(
                            ps1[kt][:],
                            u_t[dt][:, kt * P:(kt + 1) * P],
                            ht_t[dt][:, ih * NH:(ih + 1) * NH],
                            start=(dt == 0),
                            stop=(dt == DO - 1),
                        )
                for kt in range(DO):
                    copy(t1t_t[kt][:, ih * NH:(ih + 1) * NH], ps1[kt][:])

            # ---------- s_head / s_dep rows: all PE matmuls back-to-back ----
            ps_r = []
            for ih in range(2):
                ps_ri = psp.tile([P, NH], F32, tag="ps", name=f"ps_r{ih}")
                for dt in range(DO):
                    nc.tensor.matmul(
                        ps_ri[0:1, :],
                        wc_sb[:, dt:dt + 1],
                        ht_t[dt][:, ih * NH:(ih + 1) * NH],
                        start=(dt == 0),
                        stop=(dt == DO - 1),
                    )
                nc.vector.tensor_scalar(
                    row_sb[0:1, ih * NH:(ih + 1) * NH],
                    ps_ri[0:1, :], b_sb[0:1, 0:1], None, ADD,
                )
                ps_r.append(ps_ri)
            for jh in range(2):
                ps_d = psp.tile([P, NH], F32, tag="ps", name=f"ps_d{jh}")
                for kt in range(DO):
                    nc.tensor.matmul(
                        ps_d[0:1, :],
                        wc_sb[:, DO + kt:DO + kt + 1],
                        pt_t[kt][:, jh * NH:(jh + 1) * NH],
                        start=(kt == 0),
                        stop=(kt == DO - 1),
                    )
                nc.vector.tensor_copy(
                    drow_sb[0:1, jh * NH:(jh + 1) * NH], ps_d[0:1, :]
                )
                nc.gpsimd.partition_broadcast(
                    sdep_full[:, jh * NH:(jh + 1) * NH],
                    drow_sb[0:1, jh * NH:(jh + 1) * NH],
                )
            # s_head row -> per-partition column (8 tiny PE transposes)
            ps_c = psp.tile([P, NH], F32, tag="ps")
            for it in range(DO):
                nc.tensor.transpose(
                    ps_c[:, it:it + 1],
                    row_sb[0:1, it * P:(it + 1) * P],
                    one_sb[0:1, 0:1],
                )
            nc.scalar.copy(shead_col[:], ps_c[:, 0:DO])

            # ---------- mm2 + epilogue --------------------------------------
            def mm2_group(it, jh, c0, c1):
                ps = psp.tile([P, c1 - c0], F32, tag="ps", name=f"mm2_{it}_{jh}")
                for kt in range(DO):
                    nc.tensor.matmul(
                        ps[:],
                        t1t_t[kt][:, it * P:(it + 1) * P],
                        pt_t[kt][:, jh * NH + c0:jh * NH + c1],
                        start=(kt == 0),
                        stop=(kt == DO - 1),
                    )
                ot = outp.tile([P, c1 - c0], BF16, tag="out", name=f"ot_{it}_{jh}_{c0}")
                nc.vector.scalar_tensor_tensor(
                    out=ot[:], in0=ps[:],
                    scalar=shead_col[:, it:it + 1],
                    in1=sdep_full[:, jh * NH + c0:jh * NH + c1],
                    op0=ADD, op1=ADD,
                )
                nc.sync.dma_start(
                    out[it * P:(it + 1) * P, jh * NH + c0:jh * NH + c1], ot[:]
                )

            for jh in range(2):
                for it in range(DO):
                    if jh == 1 and it == DO - 1:
                        # split the final group so the tail chain is short
                        mm2_group(it, jh, 0, NH // 2)
                        mm2_group(it, jh, NH // 2, NH)
                    else:
                        mm2_group(it, jh, 0, NH)

    nc.compile()
    return nc


def _get_nc(nwarm=NWARM):
    key = ("nc", nwarm)
    if key not in _CACHE:
        _CACHE[key] = build_nc(nwarm)
    return _CACHE[key]


def _in_maps(head, dep, edge_U, edge_W, edge_b):
    bf16 = ml_dtypes.bfloat16
    head = np.asarray(head, dtype=np.float32)
    dep = np.asarray(dep, dtype=np.float32)
    identb = np.eye(P, dtype=bf16)
    u_prep = np.ascontiguousarray(
        np.asarray(edge_U, dtype=np.float32)
    ).astype(bf16).reshape(DO, P, D)
    w = np.asarray(edge_W, dtype=np.float32).reshape(-1)
    w1c = w[:D].reshape(DO, P).T
    w2c = w[D:].reshape(DO, P).T
    wc = np.ascontiguousarray(np.concatenate([w1c, w2c], axis=1)).astype(bf16)
    b0 = np.asarray(edge_b, dtype=np.float32).reshape(1, 1)
    head_b = head.astype(bf16)
    dep_b = dep.astype(bf16)
    maps = []
    for b in range(B):
        maps.append({
            "identb": identb,
            "ht": np.ascontiguousarray(head_b[b].T).reshape(DO, P, S),
            "pt": np.ascontiguousarray(dep_b[b].T).reshape(DO, P, S),
            "u": u_prep,
            "wc": wc,
            "bias0": b0,
        })
    return maps


def kernel(head, dep, edge_U, edge_W, edge_b, **run_kwargs):
    nc = _get_nc()
    maps = _in_maps(head, dep, edge_U, edge_W, edge_b)
    res = run_bass_kernel_spmd(nc, maps, core_ids=list(range(B)), **run_kwargs)
    out = np.stack(
        [np.asarray(res.results[c]["out"]).astype(np.float32) for c in range(B)],
        axis=0,
    )
    if run_kwargs:
        _CACHE["last_result"] = res
    return out
